# revision 24
# baseline (speedup 1.0000x reference)
"""BEiT-style windowed attention block on 8 Trainium2 NeuronCores.

Reference computation (per batch b, head h):
    qkv = x @ qkv_w.T + [q_bias, 0, v_bias]          # [B, N, 3C]
    q, k, v = split(qkv)                              # [B, H, N, D]
    s = (q * D**-0.5) @ k.T + rpb_table[rel_idx].T    # [B, H, N, N]
    p = softmax(s, axis=-1)
    out = (p @ v).reshape(B, N, C) @ proj_w.T + proj_b

Sharding: pure data parallel — batch 64 split as 8 batches per core,
weights + rel-pos-bias table replicated. No collectives.

Device-side layout strategy (per core):
  - x is staged host-side as x^T ("f-major": feature on partitions) so the
    qkv matmuls can use it as the moving operand directly.
  - q^T, k^T are produced f-major ([feat, token]) so the per-head attention
    matmul s^T[m, n] = k^T.T @ q^T needs no transposes.  Softmax runs over
    the partition (m) axis: exp on ACT, denominators via ones-column
    matmuls on the PE, division via a reciprocal row broadcast (DRAM-bounce
    DMA) — softmax is shift-invariant and the scores here are O(1), so the
    max-subtraction is skipped.
  - v is produced token-major ([token, feat]) which is exactly the lhsT
    layout stage-3 (p @ v) wants; its output comes out f-major, which is
    exactly the lhsT layout the final projection wants; the projection
    output comes out token-major, which is what the DMA back to HBM wants.
  - head pairs sit at partition offsets 0/64, so the K=64 / M=64 attention
    matmuls auto-pack into distinct PE row/col groups and run concurrently.
"""

import sys

sys.path.insert(0, "/opt/trn_rl_repo")

import numpy as np

import concourse.bass as bass
import concourse.mybir as mybir
import concourse.tile as tile
from concourse import bacc
from concourse.bass_utils import run_bass_kernel_spmd

F32 = mybir.dt.float32
# Matmul operand dtype. fp16 streams at 1 row/cycle (4x plain fp32's LOW_HIGH
# double-pass), keeps the PE HAM clock warm (unlike float32r, whose datapath
# doesn't register as PE activity and re-throttles the clock to 1.2 GHz), and
# carries 3 more mantissa bits than bf16. All values here are O(100) at most,
# far from fp16 range limits. PSUM accumulation and softmax arithmetic stay
# fp32.
DT_BIG = mybir.dt.float16
DT_ATT = mybir.dt.float16

DIM = 768
H = 12
D = 64
N = 197  # tokens per image
B = 64
CORES = 8
BSH = B // CORES  # batches per core
KO = DIM // 128  # contraction subtiles
SCALE = D ** -0.5
N0, N1 = 128, N - 128  # token chunk sizes (128, 69)


def _act_raw(nc, out, in_, func):
    """InstActivation without bass's accuracy blocklist (used for the
    table-based Reciprocal: measured ~1e-5 rel err, far below the fp16
    matmul noise floor)."""
    eng = nc.scalar
    ins = [eng.lower_ap(in_)]
    for arg in (0.0, 1.0, 0.0):
        ins.append(mybir.ImmediateValue(dtype=mybir.dt.float32, value=arg))
    return eng.add_instruction(
        mybir.InstActivation(
            name=nc.get_next_instruction_name(),
            func=func,
            ins=ins,
            outs=[eng.lower_ap(out)],
        )
    )


def build_program(n_batches: int = BSH):
    nc = bacc.Bacc("TRN2", target_bir_lowering=False, debug=False, num_devices=CORES)

    T = n_batches * N
    xt_d = nc.dram_tensor("xt", [128, KO, T], DT_BIG, kind="ExternalInput")
    qkw_d = nc.dram_tensor("qkw", [12, 128, KO, 128], DT_BIG, kind="ExternalInput")
    vw_d = nc.dram_tensor("vw", [128, KO, DIM], DT_BIG, kind="ExternalInput")
    pw_d = nc.dram_tensor("pw", [128, KO, DIM], DT_BIG, kind="ExternalInput")
    # rel-pos-bias, transposed: bias[mo, mi, h, n] = rpb[rel_idx[n, m], h]
    ebias_d = nc.dram_tensor("ebias", [2, 128, H, N], F32, kind="ExternalInput")
    qb_d = nc.dram_tensor("qb", [128, 12], F32, kind="ExternalInput")
    vb_d = nc.dram_tensor("vb", [1, DIM], F32, kind="ExternalInput")
    pb_d = nc.dram_tensor("pb", [1, DIM], F32, kind="ExternalInput")
    onesw_d = nc.dram_tensor("onesw", [128, 64], DT_ATT, kind="ExternalInput")

    out_d = nc.dram_tensor("out", [n_batches, N, DIM], F32, kind="ExternalOutput")

    with tile.TileContext(nc) as tc:
        with (
            tc.tile_pool(name="wpool", bufs=1) as wpool,
            tc.tile_pool(name="xpool", bufs=2) as xpool,
            tc.tile_pool(name="qkpool", bufs=2) as qkpool,
            tc.tile_pool(name="vpool", bufs=2) as vpool,
            tc.tile_pool(name="eras", bufs=3) as eras,
            tc.tile_pool(name="epool", bufs=3) as epool,
            tc.tile_pool(name="opool", bufs=2) as opool,
            tc.tile_pool(name="otpool", bufs=2) as otpool,
            tc.tile_pool(name="rpool", bufs=2) as rpool,
            tc.tile_pool(name="outpool", bufs=2) as outpool,
            tc.tile_pool(name="ps_mm", bufs=2, space="PSUM") as ps_mm,
            tc.tile_pool(name="ps_s", bufs=3, space="PSUM") as ps_s,
            tc.tile_pool(name="ps_pd", bufs=3, space="PSUM") as ps_pd,
            tc.tile_pool(name="dscr", bufs=2, space="DRAM") as dscr,
        ):
            # ---- persistent weights ----
            qkw = []
            for ft in range(12):
                t = wpool.tile([128, KO, 128], DT_BIG, tag=f"qkw{ft}")
                nc.sync.dma_start(t[:], qkw_d[ft])
                qkw.append(t)
            vw = wpool.tile([128, KO, DIM], DT_BIG, tag="vw")
            nc.sync.dma_start(vw[:], vw_d[:])
            pw = wpool.tile([128, KO, DIM], DT_BIG, tag="pw")
            nc.sync.dma_start(pw[:], pw_d[:])
            ebias = wpool.tile([128, 2, H, N], F32, tag="ebias")
            for mo in range(2):
                nc.sync.dma_start(ebias[:, mo], ebias_d[mo])
            qb = wpool.tile([128, 12], F32, tag="qb")
            nc.sync.dma_start(qb[:], qb_d[:])
            # per-feature biases broadcast across partitions (stride-0 DMA)
            vb_bc = wpool.tile([128, DIM], F32, tag="vb_bc")
            nc.sync.dma_start(
                vb_bc[:],
                bass.AP(tensor=vb_d.ap().tensor, offset=vb_d.ap().offset,
                        ap=[[0, 128]] + list(vb_d.ap().ap[1:])),
            )
            pb_bc = wpool.tile([128, DIM], F32, tag="pb_bc")
            nc.sync.dma_start(
                pb_bc[:],
                bass.AP(tensor=pb_d.ap().tensor, offset=pb_d.ap().offset,
                        ap=[[0, 128]] + list(pb_d.ap().ap[1:])),
            )
            onesw = wpool.tile([128, 64], DT_ATT, tag="onesw")
            nc.sync.dma_start(onesw[:], onesw_d[:])


            assert n_batches % 2 == 0
            for chunk in range(n_batches // 2):
                # ---- load x^T for a 2-batch chunk ----
                xt = xpool.tile([128, KO, 2 * N], DT_BIG, tag="xt")
                nc.sync.dma_start(xt[:], xt_d[:, :, 2 * N * chunk : 2 * N * (chunk + 1)])

                # ---- q^T / k^T (f-major), both batches at once (N=394) ----
                # 456 = 2N + 62 pad cols so the mo=1 score matmul can use a
                # full M=128 stationary slice (rows 69:128 produce scores of
                # neighbouring tokens, initialized but unused).
                qkT = qkpool.tile([128, 12, 456], DT_ATT, tag="qkT")
                nc.vector.memset(qkT[:, :, 2 * N : 456], 0.0)
                for ft in range(12):
                    ps = ps_mm.tile([128, 512], F32, tag="mm")
                    for ko in range(KO):
                        nc.tensor.matmul(
                            ps[:, 0 : 2 * N],
                            qkw[ft][:, ko],
                            xt[:, ko],
                            start=(ko == 0),
                            stop=(ko == KO - 1),
                        )
                    nc.scalar.activation(
                        qkT[:, ft, 0 : 2 * N],
                        ps[:, 0 : 2 * N],
                        mybir.ActivationFunctionType.Identity,
                        bias=qb[:, ft : ft + 1],
                        scale=SCALE if ft < 6 else 1.0,
                    )

                for i in range(2):
                    b = 2 * chunk + i
                    boff = i * N

                    # ---- v (token-major), augmented per head with a ones
                    # half so stage 3 computes output and denominator in one
                    # M=128 matmul.  Even heads: [v | 1], odd heads: [1 | v].
                    v_sb = vpool.tile([128, 2, H, 128], DT_ATT, tag="v")
                    for par in range(2):
                        dst = v_sb.rearrange("p a (g two) c -> p a g two c", two=2)[
                            :, :, :, par, 64 * (1 - par) : 64 * (1 - par) + 64
                        ]
                        osrc = bass.AP(
                            tensor=onesw.tensor, offset=onesw.offset,
                            ap=[list(onesw.ap[0])]
                            + [[0, 2], [0, H // 2]]
                            + [list(onesw.ap[1])],
                        )
                        nc.vector.tensor_copy(dst, osrc)
                    for no, tw in ((0, N0), (1, N1)):
                        for fo, fw in ((0, 512), (512, 256)):
                            psv = ps_mm.tile([128, 512], F32, tag="mm")
                            for ko in range(KO):
                                nc.tensor.matmul(
                                    psv[0:tw, 0:fw],
                                    xt[:, ko, boff + no * 128 : boff + no * 128 + tw],
                                    vw[:, ko, fo : fo + fw],
                                    start=(ko == 0),
                                    stop=(ko == KO - 1),
                                )
                            g = fw // 128
                            hb = fo // 64
                            vsrc = psv[0:tw, 0:fw].rearrange(
                                "p (g two d) -> p g two d", two=2, d=64
                            )
                            vbs = vb_bc[0:tw, fo : fo + fw].rearrange(
                                "p (g two d) -> p g two d", two=2, d=64
                            )
                            vdst = v_sb[0:tw, no, hb : hb + 2 * g, :].rearrange(
                                "p (g two) c -> p g two c", two=2
                            )
                            for par in range(2):
                                nc.vector.tensor_add(
                                    vdst[:, :, par, 64 * par : 64 * par + 64],
                                    vsrc[:, :, par, :],
                                    vbs[:, :, par, :],
                                )

                    # ---- attention, head pairs (2j, 2j+1) ----
                    # The two heads of a pair sit at partition 0 / 64 in the
                    # f-major layouts, so their matmuls land in different PE
                    # row/col groups and run concurrently when adjacent.
                    ohT = opool.tile([128, KO, N], DT_BIG, tag="ohT")
                    out_all = otpool.tile([128, H // 2, N], F32, tag="out_all",
                                          name="out_all")
                    den_cmp = otpool.tile([128, 3, N], F32, tag="den_cmp",
                                          name="den_cmp")
                    nc.vector.memset(den_cmp[:], 1.0)
                    for j in range(H // 2):
                        hA, hB = 2 * j, 2 * j + 1
                        qkTA = qkT[0:64, :, :]
                        qkTB = qkT[64:128, :, :]
                        pssA = ps_s.tile([128, 512], F32, tag="s", name="pssA")
                        pssB = ps_s.tile([128, 512], F32, tag="s", name="pssB")
                        for mo in range(2):
                            psl = slice(mo * N, mo * N + N)
                            nc.tensor.matmul(
                                pssA[:, psl],
                                qkTA[:, 6 + j, boff + 128 * mo : boff + 128 * mo + 128],
                                qkTA[:, j, boff : boff + N],
                                start=True, stop=True,
                            )
                            nc.tensor.matmul(
                                pssB[:, psl],
                                qkTB[:, 6 + j, boff + 128 * mo : boff + 128 * mo + 128],
                                qkTB[:, j, boff : boff + N],
                                start=True, stop=True,
                            )
                        # one DVE add drains the score psum (bias applied,
                        # fp16 out), then exp runs from SBUF on ACT.
                        es_pair = []
                        for pss, h in ((pssA, hA), (pssB, hB)):
                            sa = eras.tile([128, 2, N], DT_ATT, tag="sa")
                            nc.vector.tensor_add(
                                sa[:],
                                pss[:, 0 : 2 * N].rearrange("p (a n) -> p a n", a=2),
                                ebias[:, :, h, :],
                            )
                            es = epool.tile([128, 2, N], DT_ATT, tag="es")
                            nc.scalar.activation(
                                es.rearrange("p a n -> p (a n)"),
                                sa.rearrange("p a n -> p (a n)"),
                                mybir.ActivationFunctionType.Exp,
                            )
                            es_pair.append(es)
                        esA, esB = es_pair

                        # stage 3: one M=128 matmul per head gives out^T rows
                        # and 64 replicated denominator rows in one group.
                        pdA = ps_pd.tile([128, 512], F32, tag="pd", name="pdA")
                        pdB = ps_pd.tile([128, 512], F32, tag="pd", name="pdB")
                        nc.tensor.matmul(pdA[:, 0:N], v_sb[:, 0, hA, :], esA[:, 0, :],
                                         start=True, stop=False)
                        nc.tensor.matmul(pdB[:, 0:N], v_sb[:, 0, hB, :], esB[:, 0, :],
                                         start=True, stop=False)
                        nc.tensor.matmul(pdA[:, 0:N], v_sb[0:N1, 1, hA, :],
                                         esA[0:N1, 1, :], start=False, stop=True)
                        nc.tensor.matmul(pdB[:, 0:N], v_sb[0:N1, 1, hB, :],
                                         esB[0:N1, 1, :], start=False, stop=True)

                        # even head: out rows 0:64, den (replicated) 64:128;
                        # odd head: den rows 0:64, out rows 64:128.
                        nc.scalar.activation(
                            out_all[0:64, j, :], pdA[0:64, 0:N],
                            mybir.ActivationFunctionType.Copy,
                        )
                        nc.scalar.activation(
                            out_all[64:128, j, :], pdB[64:128, 0:N],
                            mybir.ActivationFunctionType.Copy,
                        )
                        eb = 64 + 32 * (j % 2)
                        ob2 = 32 * (j % 2)
                        nc.scalar.activation(
                            den_cmp[eb : eb + 1, j // 2, :], pdA[eb : eb + 1, 0:N],
                            mybir.ActivationFunctionType.Copy,
                        )
                        nc.scalar.activation(
                            den_cmp[ob2 : ob2 + 1, j // 2, :], pdB[ob2 : ob2 + 1, 0:N],
                            mybir.ActivationFunctionType.Copy,
                        )

                    # softmax division: one compact reciprocal per batch, then
                    # broadcast each head's row over its 64 output partitions
                    # via a DRAM bounce, and one elementwise multiply.
                    rvc = rpool.tile([128, 3, N], F32, tag="rvc")
                    nc.vector.reciprocal(rvc[:], den_cmp[:])
                    scr = dscr.tile([4, 3, N], F32, tag="scr")
                    for bi, base in enumerate((0, 32, 64, 96)):
                        nc.sync.dma_start(scr[bi : bi + 1], rvc[base : base + 1, :, :])
                    rv2 = rpool.tile([128, H // 2, N], F32, tag="rv2")
                    for j in range(H // 2):
                        sA_ap = scr[2 + j % 2, j // 2, :][None]
                        sB_ap = scr[j % 2, j // 2, :][None]
                        for rows, s_ap in ((slice(0, 64), sA_ap),
                                           (slice(64, 128), sB_ap)):
                            bsrc = bass.AP(
                                tensor=s_ap.tensor, offset=s_ap.offset,
                                ap=[[0, 64]] + list(s_ap.ap[1:]),
                            )
                            nc.sync.dma_start(rv2[rows, j, :], bsrc)
                    nc.vector.tensor_mul(ohT[:], out_all[:], rv2[:])

                    # ---- projection (token-major out) + bias ----
                    out_sb = outpool.tile([128, 2, DIM], F32, tag="out")
                    for no, tw in ((0, N0), (1, N1)):
                        for fo, fw in ((0, 512), (512, 256)):
                            psp = ps_mm.tile([128, 512], F32, tag="mm")
                            for ko in range(KO):
                                nc.tensor.matmul(
                                    psp[0:tw, 0:fw],
                                    ohT[:, ko, no * 128 : no * 128 + tw],
                                    pw[:, ko, fo : fo + fw],
                                    start=(ko == 0),
                                    stop=(ko == KO - 1),
                                )
                            nc.vector.tensor_add(
                                out_sb[0:tw, no, fo : fo + fw],
                                psp[0:tw, 0:fw],
                                pb_bc[0:tw, fo : fo + fw],
                            )
                    nc.sync.dma_start(out_d[b, 0:128, :], out_sb[:, 0, :])
                    nc.sync.dma_start(out_d[b, 128:N, :], out_sb[0:N1, 1, :])

    nc.compile()
    return nc


def _np_dt(dt):
    import ml_dtypes

    return {
        mybir.dt.float32: np.float32,
        mybir.dt.float32r: np.float32,
        mybir.dt.float16: np.float16,
        mybir.dt.bfloat16: ml_dtypes.bfloat16,
    }[dt]


def prep_inputs(x, qkv_w, q_bias, v_bias, rpb_table, proj_w, proj_b, rel_idx):
    """Host-side staging: shard x over cores, lay out weights for SBUF."""
    x = np.asarray(x, dtype=np.float32)
    qkv_w = np.asarray(qkv_w, dtype=np.float32)
    proj_w = np.asarray(proj_w, dtype=np.float32)
    q_bias = np.asarray(q_bias, dtype=np.float32)
    v_bias = np.asarray(v_bias, dtype=np.float32)
    rpb_table = np.asarray(rpb_table, dtype=np.float32)
    proj_b = np.asarray(proj_b, dtype=np.float32)
    rel_idx = np.asarray(rel_idx)

    big = _np_dt(DT_BIG)
    qkvwT = np.ascontiguousarray(qkv_w.T)  # [768, 2304]
    qkw = np.ascontiguousarray(
        qkvwT[:, : 2 * DIM].reshape(KO, 128, 12, 128).transpose(2, 1, 0, 3)
    ).astype(big)
    vw = np.ascontiguousarray(
        qkvwT[:, 2 * DIM :].reshape(KO, 128, DIM).transpose(1, 0, 2)
    ).astype(big)
    pw = np.ascontiguousarray(
        proj_w.T.reshape(KO, 128, DIM).transpose(1, 0, 2)
    ).astype(big)

    # ebias[mo, mi, h, n] = exp(rpb_table[rel_idx[n, m], h]) with m = mo*128+mi
    bnm = rpb_table[rel_idx]  # [n, m, H]
    bias = np.zeros((2 * 128, H, N), dtype=np.float32)
    bias[:N] = bnm.transpose(1, 2, 0)  # [m, H, n]
    bias = bias.reshape(2, 128, H, N)

    qb = np.zeros((128, 12), dtype=np.float32)
    qb[:, :6] = (q_bias * SCALE).reshape(KO, 128).T
    vb = np.ascontiguousarray(v_bias[None, :])
    pb = np.ascontiguousarray(proj_b[None, :])

    import ml_dtypes

    shared = {
        "qkw": qkw, "vw": vw, "pw": pw, "ebias": np.ascontiguousarray(bias),
        "qb": qb, "vb": vb, "pb": pb,
        "onesw": np.ones((128, 64), dtype=_np_dt(DT_ATT)),
    }
    in_maps = []
    for c in range(CORES):
        xs = x[c * BSH : (c + 1) * BSH]  # [BSH, N, DIM]
        xt = np.ascontiguousarray(
            xs.reshape(BSH * N, DIM).T.reshape(KO, 128, BSH * N).transpose(1, 0, 2)
        ).astype(big)
        in_maps.append({"xt": xt, **shared})
    return in_maps


def _ensure_ntff_hook():
    """Register the axon NTFF profile hook so trace=True yields exec_time_ns.

    The image's antenv package lacks axon_hooks, so boot() degrades silently;
    supply the module via sys.modules and re-register the ctypes hook.
    Best-effort: failure only disables tracing, not execution."""
    import types

    if "antenv.axon_hooks" in sys.modules:
        return
    try:
        mod = types.ModuleType("antenv.axon_hooks")
        _hook = [None]
        mod.set_axon_ntff_profile_hook = lambda h: _hook.__setitem__(0, h)
        mod.get_axon_ntff_profile_hook = lambda: _hook[0]
        from trn_agent_boot.trn_boot import _ntff_profile_via_ctypes

        mod.set_axon_ntff_profile_hook(
            _ntff_profile_via_ctypes("/opt/axon/libaxon_pjrt.so")
        )
        sys.modules["antenv.axon_hooks"] = mod
    except Exception:
        pass


_NC = None


def _get_nc():
    global _NC
    if _NC is None:
        _NC = build_program(BSH)
    return _NC


def kernel(x, qkv_w, q_bias, v_bias, rpb_table, proj_w, proj_b, rel_idx,
           _trace=False, **trace_kwargs):
    if _trace:
        _ensure_ntff_hook()
    nc = _get_nc()
    in_maps = prep_inputs(x, qkv_w, q_bias, v_bias, rpb_table, proj_w, proj_b, rel_idx)
    res = run_bass_kernel_spmd(
        nc, in_maps, core_ids=list(range(CORES)), trace=_trace, **trace_kwargs
    )
    out = np.concatenate([res.results[c]["out"] for c in range(CORES)], axis=0)
    if _trace:
        return out, res
    return out


# revision 25
# speedup vs baseline: 1.0312x; 1.0312x over previous
"""BEiT-style windowed attention block on 8 Trainium2 NeuronCores.

Reference computation (per batch b, head h):
    qkv = x @ qkv_w.T + [q_bias, 0, v_bias]          # [B, N, 3C]
    q, k, v = split(qkv)                              # [B, H, N, D]
    s = (q * D**-0.5) @ k.T + rpb_table[rel_idx].T    # [B, H, N, N]
    p = softmax(s, axis=-1)
    out = (p @ v).reshape(B, N, C) @ proj_w.T + proj_b

Sharding: pure data parallel — batch 64 split as 8 batches per core,
weights + rel-pos-bias table replicated. No collectives.

Device-side layout strategy (per core):
  - x is staged host-side as x^T ("f-major": feature on partitions) so the
    qkv matmuls can use it as the moving operand directly.
  - q^T, k^T are produced f-major ([feat, token]) so the per-head attention
    matmul s^T[m, n] = k^T.T @ q^T needs no transposes.  Softmax runs over
    the partition (m) axis: exp on ACT, denominators via ones-column
    matmuls on the PE, division via a reciprocal row broadcast (DRAM-bounce
    DMA) — softmax is shift-invariant and the scores here are O(1), so the
    max-subtraction is skipped.
  - v is produced token-major ([token, feat]) which is exactly the lhsT
    layout stage-3 (p @ v) wants; its output comes out f-major, which is
    exactly the lhsT layout the final projection wants; the projection
    output comes out token-major, which is what the DMA back to HBM wants.
  - head pairs sit at partition offsets 0/64, so the K=64 / M=64 attention
    matmuls auto-pack into distinct PE row/col groups and run concurrently.
"""

import sys

sys.path.insert(0, "/opt/trn_rl_repo")

import numpy as np

import concourse.bass as bass
import concourse.mybir as mybir
import concourse.tile as tile
from concourse import bacc
from concourse.bass_utils import run_bass_kernel_spmd

F32 = mybir.dt.float32
# Matmul operand dtype. fp16 streams at 1 row/cycle (4x plain fp32's LOW_HIGH
# double-pass), keeps the PE HAM clock warm (unlike float32r, whose datapath
# doesn't register as PE activity and re-throttles the clock to 1.2 GHz), and
# carries 3 more mantissa bits than bf16. All values here are O(100) at most,
# far from fp16 range limits. PSUM accumulation and softmax arithmetic stay
# fp32.
DT_BIG = mybir.dt.float16
DT_ATT = mybir.dt.float16

DIM = 768
H = 12
D = 64
N = 197  # tokens per image
B = 64
CORES = 8
BSH = B // CORES  # batches per core
KO = DIM // 128  # contraction subtiles
SCALE = D ** -0.5
N0, N1 = 128, N - 128  # token chunk sizes (128, 69)


def _act_raw(nc, out, in_, func):
    """InstActivation without bass's accuracy blocklist (used for the
    table-based Reciprocal: measured ~1e-5 rel err, far below the fp16
    matmul noise floor)."""
    eng = nc.scalar
    ins = [eng.lower_ap(in_)]
    for arg in (0.0, 1.0, 0.0):
        ins.append(mybir.ImmediateValue(dtype=mybir.dt.float32, value=arg))
    return eng.add_instruction(
        mybir.InstActivation(
            name=nc.get_next_instruction_name(),
            func=func,
            ins=ins,
            outs=[eng.lower_ap(out)],
        )
    )


def build_program(n_batches: int = BSH):
    nc = bacc.Bacc("TRN2", target_bir_lowering=False, debug=False, num_devices=CORES)

    T = n_batches * N
    xt_d = nc.dram_tensor("xt", [128, KO, T], DT_BIG, kind="ExternalInput")
    qkw_d = nc.dram_tensor("qkw", [12, 128, KO, 128], DT_BIG, kind="ExternalInput")
    vw_d = nc.dram_tensor("vw", [128, KO, DIM], DT_BIG, kind="ExternalInput")
    pw_d = nc.dram_tensor("pw", [128, KO, DIM], DT_BIG, kind="ExternalInput")
    # rel-pos-bias, transposed: bias[mo, mi, h, n] = rpb[rel_idx[n, m], h]
    ebias_d = nc.dram_tensor("ebias", [2, 128, H, N], F32, kind="ExternalInput")
    qb_d = nc.dram_tensor("qb", [128, 12], F32, kind="ExternalInput")
    vb_d = nc.dram_tensor("vb", [1, DIM], F32, kind="ExternalInput")
    pb_d = nc.dram_tensor("pb", [1, DIM], F32, kind="ExternalInput")
    onesw_d = nc.dram_tensor("onesw", [128, 64], DT_ATT, kind="ExternalInput")

    out_d = nc.dram_tensor("out", [n_batches, N, DIM], F32, kind="ExternalOutput")

    with tile.TileContext(nc) as tc:
        with (
            tc.tile_pool(name="wpool", bufs=1) as wpool,
            tc.tile_pool(name="xpool", bufs=2) as xpool,
            tc.tile_pool(name="qkpool", bufs=2) as qkpool,
            tc.tile_pool(name="vpool", bufs=2) as vpool,
            tc.tile_pool(name="eras", bufs=3) as eras,
            tc.tile_pool(name="epool", bufs=3) as epool,
            tc.tile_pool(name="opool", bufs=2) as opool,
            tc.tile_pool(name="otpool", bufs=2) as otpool,
            tc.tile_pool(name="rpool", bufs=2) as rpool,
            tc.tile_pool(name="outpool", bufs=2) as outpool,
            tc.tile_pool(name="ps_mm", bufs=4, space="PSUM") as ps_mm,
            tc.tile_pool(name="ps_s", bufs=2, space="PSUM") as ps_s,
            tc.tile_pool(name="ps_pd", bufs=2, space="PSUM") as ps_pd,
            tc.tile_pool(name="dscr", bufs=2, space="DRAM") as dscr,
        ):
            # ---- persistent weights ----
            qkw = []
            for ft in range(12):
                t = wpool.tile([128, KO, 128], DT_BIG, tag=f"qkw{ft}")
                nc.sync.dma_start(t[:], qkw_d[ft])
                qkw.append(t)
            vw = wpool.tile([128, KO, DIM], DT_BIG, tag="vw")
            nc.sync.dma_start(vw[:], vw_d[:])
            pw = wpool.tile([128, KO, DIM], DT_BIG, tag="pw")
            nc.sync.dma_start(pw[:], pw_d[:])
            ebias = wpool.tile([128, 2, H, N], F32, tag="ebias")
            for mo in range(2):
                nc.sync.dma_start(ebias[:, mo], ebias_d[mo])
            qb = wpool.tile([128, 12], F32, tag="qb")
            nc.sync.dma_start(qb[:], qb_d[:])
            # per-feature biases broadcast across partitions (stride-0 DMA)
            vb_bc = wpool.tile([128, DIM], F32, tag="vb_bc")
            nc.sync.dma_start(
                vb_bc[:],
                bass.AP(tensor=vb_d.ap().tensor, offset=vb_d.ap().offset,
                        ap=[[0, 128]] + list(vb_d.ap().ap[1:])),
            )
            pb_bc = wpool.tile([128, DIM], F32, tag="pb_bc")
            nc.sync.dma_start(
                pb_bc[:],
                bass.AP(tensor=pb_d.ap().tensor, offset=pb_d.ap().offset,
                        ap=[[0, 128]] + list(pb_d.ap().ap[1:])),
            )
            onesw = wpool.tile([128, 64], DT_ATT, tag="onesw")
            nc.sync.dma_start(onesw[:], onesw_d[:])


            assert n_batches % 2 == 0
            for chunk in range(n_batches // 2):
                # ---- load x^T for a 2-batch chunk ----
                xt = xpool.tile([128, KO, 2 * N], DT_BIG, tag="xt")
                nc.sync.dma_start(xt[:], xt_d[:, :, 2 * N * chunk : 2 * N * (chunk + 1)])

                # ---- q^T / k^T (f-major), both batches at once (N=394) ----
                # 456 = 2N + 62 pad cols so the mo=1 score matmul can use a
                # full M=128 stationary slice (rows 69:128 produce scores of
                # neighbouring tokens, initialized but unused).
                qkT = qkpool.tile([128, 12, 456], DT_ATT, tag="qkT")
                nc.vector.memset(qkT[:, :, 2 * N : 456], 0.0)
                for ft in range(12):
                    ps = ps_mm.tile([128, 512], F32, tag="mm")
                    for ko in range(KO):
                        nc.tensor.matmul(
                            ps[:, 0 : 2 * N],
                            qkw[ft][:, ko],
                            xt[:, ko],
                            start=(ko == 0),
                            stop=(ko == KO - 1),
                        )
                    nc.scalar.activation(
                        qkT[:, ft, 0 : 2 * N],
                        ps[:, 0 : 2 * N],
                        mybir.ActivationFunctionType.Identity,
                        bias=qb[:, ft : ft + 1],
                        scale=SCALE if ft < 6 else 1.0,
                    )

                for i in range(2):
                    b = 2 * chunk + i
                    boff = i * N

                    # ---- v (token-major), augmented per head with a ones
                    # half so stage 3 computes output and denominator in one
                    # M=128 matmul.  Even heads: [v | 1], odd heads: [1 | v].
                    v_sb = vpool.tile([128, 2, H, 128], DT_ATT, tag="v")
                    for par in range(2):
                        dst = v_sb.rearrange("p a (g two) c -> p a g two c", two=2)[
                            :, :, :, par, 64 * (1 - par) : 64 * (1 - par) + 64
                        ]
                        osrc = bass.AP(
                            tensor=onesw.tensor, offset=onesw.offset,
                            ap=[list(onesw.ap[0])]
                            + [[0, 2], [0, H // 2]]
                            + [list(onesw.ap[1])],
                        )
                        nc.vector.tensor_copy(dst, osrc)
                    for no, tw in ((0, N0), (1, N1)):
                        for fo, fw in ((0, 512), (512, 256)):
                            psv = ps_mm.tile([128, 512], F32, tag="mm")
                            for ko in range(KO):
                                nc.tensor.matmul(
                                    psv[0:tw, 0:fw],
                                    xt[:, ko, boff + no * 128 : boff + no * 128 + tw],
                                    vw[:, ko, fo : fo + fw],
                                    start=(ko == 0),
                                    stop=(ko == KO - 1),
                                )
                            g = fw // 128
                            hb = fo // 64
                            vsrc = psv[0:tw, 0:fw].rearrange(
                                "p (g two d) -> p g two d", two=2, d=64
                            )
                            vbs = vb_bc[0:tw, fo : fo + fw].rearrange(
                                "p (g two d) -> p g two d", two=2, d=64
                            )
                            vdst = v_sb[0:tw, no, hb : hb + 2 * g, :].rearrange(
                                "p (g two) c -> p g two c", two=2
                            )
                            for par in range(2):
                                nc.vector.tensor_add(
                                    vdst[:, :, par, 64 * par : 64 * par + 64],
                                    vsrc[:, :, par, :],
                                    vbs[:, :, par, :],
                                )

                    # ---- attention, head pairs (2j, 2j+1) ----
                    # The two heads of a pair sit at partition 0 / 64 in the
                    # f-major layouts, so their matmuls land in different PE
                    # row/col groups and run concurrently when adjacent.
                    ohT = opool.tile([128, KO, N], DT_BIG, tag="ohT")
                    out_all = otpool.tile([128, H // 2, N], F32, tag="out_all",
                                          name="out_all")
                    den_cmp = otpool.tile([128, 3, N], F32, tag="den_cmp",
                                          name="den_cmp")
                    nc.vector.memset(den_cmp[:], 1.0)
                    for j in range(H // 2):
                        hA, hB = 2 * j, 2 * j + 1
                        qkTA = qkT[0:64, :, :]
                        qkTB = qkT[64:128, :, :]
                        pssA = ps_s.tile([128, 512], F32, tag="s", name="pssA")
                        pssB = ps_s.tile([128, 512], F32, tag="s", name="pssB")
                        for mo in range(2):
                            psl = slice(mo * N, mo * N + N)
                            nc.tensor.matmul(
                                pssA[:, psl],
                                qkTA[:, 6 + j, boff + 128 * mo : boff + 128 * mo + 128],
                                qkTA[:, j, boff : boff + N],
                                start=True, stop=True,
                            )
                            nc.tensor.matmul(
                                pssB[:, psl],
                                qkTB[:, 6 + j, boff + 128 * mo : boff + 128 * mo + 128],
                                qkTB[:, j, boff : boff + N],
                                start=True, stop=True,
                            )
                        # one DVE add drains the score psum (bias applied,
                        # fp16 out), then exp runs from SBUF on ACT.
                        es_pair = []
                        for pss, h in ((pssA, hA), (pssB, hB)):
                            sa = eras.tile([128, 2, N], DT_ATT, tag="sa")
                            nc.vector.tensor_add(
                                sa[:],
                                pss[:, 0 : 2 * N].rearrange("p (a n) -> p a n", a=2),
                                ebias[:, :, h, :],
                            )
                            es = epool.tile([128, 2, N], DT_ATT, tag="es")
                            nc.scalar.activation(
                                es.rearrange("p a n -> p (a n)"),
                                sa.rearrange("p a n -> p (a n)"),
                                mybir.ActivationFunctionType.Exp,
                            )
                            es_pair.append(es)
                        esA, esB = es_pair

                        # stage 3: one M=128 matmul per head gives out^T rows
                        # and 64 replicated denominator rows in one group.
                        pdA = ps_pd.tile([128, 512], F32, tag="pd", name="pdA")
                        pdB = ps_pd.tile([128, 512], F32, tag="pd", name="pdB")
                        nc.tensor.matmul(pdA[:, 0:N], v_sb[:, 0, hA, :], esA[:, 0, :],
                                         start=True, stop=False)
                        nc.tensor.matmul(pdB[:, 0:N], v_sb[:, 0, hB, :], esB[:, 0, :],
                                         start=True, stop=False)
                        nc.tensor.matmul(pdA[:, 0:N], v_sb[0:N1, 1, hA, :],
                                         esA[0:N1, 1, :], start=False, stop=True)
                        nc.tensor.matmul(pdB[:, 0:N], v_sb[0:N1, 1, hB, :],
                                         esB[0:N1, 1, :], start=False, stop=True)

                        # even head: out rows 0:64, den (replicated) 64:128;
                        # odd head: den rows 0:64, out rows 64:128.
                        nc.scalar.activation(
                            out_all[0:64, j, :], pdA[0:64, 0:N],
                            mybir.ActivationFunctionType.Copy,
                        )
                        nc.scalar.activation(
                            out_all[64:128, j, :], pdB[64:128, 0:N],
                            mybir.ActivationFunctionType.Copy,
                        )
                        eb = 64 + 32 * (j % 2)
                        ob2 = 32 * (j % 2)
                        nc.vector.tensor_copy(
                            den_cmp[eb : eb + 1, j // 2, :], pdA[eb : eb + 1, 0:N]
                        )
                        nc.vector.tensor_copy(
                            den_cmp[ob2 : ob2 + 1, j // 2, :], pdB[ob2 : ob2 + 1, 0:N]
                        )

                    # softmax division: one compact reciprocal per batch, then
                    # broadcast each head's row over its 64 output partitions
                    # via a DRAM bounce, and one elementwise multiply.
                    rvc = rpool.tile([128, 3, N], F32, tag="rvc")
                    nc.vector.reciprocal(rvc[:], den_cmp[:])
                    scr = dscr.tile([4, 3, N], F32, tag="scr")
                    for bi, base in enumerate((0, 32, 64, 96)):
                        nc.sync.dma_start(scr[bi : bi + 1], rvc[base : base + 1, :, :])
                    rv2 = rpool.tile([128, H // 2, N], F32, tag="rv2")
                    for j in range(H // 2):
                        sA_ap = scr[2 + j % 2, j // 2, :][None]
                        sB_ap = scr[j % 2, j // 2, :][None]
                        for rows, s_ap in ((slice(0, 64), sA_ap),
                                           (slice(64, 128), sB_ap)):
                            bsrc = bass.AP(
                                tensor=s_ap.tensor, offset=s_ap.offset,
                                ap=[[0, 64]] + list(s_ap.ap[1:]),
                            )
                            nc.sync.dma_start(rv2[rows, j, :], bsrc)
                    nc.gpsimd.tensor_mul(ohT[:], out_all[:], rv2[:])

                    # ---- projection (token-major out) + bias ----
                    out_sb = outpool.tile([128, 2, DIM], F32, tag="out")
                    for no, tw in ((0, N0), (1, N1)):
                        for fo, fw in ((0, 512), (512, 256)):
                            psp = ps_mm.tile([128, 512], F32, tag="mm")
                            for ko in range(KO):
                                nc.tensor.matmul(
                                    psp[0:tw, 0:fw],
                                    ohT[:, ko, no * 128 : no * 128 + tw],
                                    pw[:, ko, fo : fo + fw],
                                    start=(ko == 0),
                                    stop=(ko == KO - 1),
                                )
                            nc.vector.tensor_add(
                                out_sb[0:tw, no, fo : fo + fw],
                                psp[0:tw, 0:fw],
                                pb_bc[0:tw, fo : fo + fw],
                            )
                    nc.sync.dma_start(out_d[b, 0:128, :], out_sb[:, 0, :])
                    nc.sync.dma_start(out_d[b, 128:N, :], out_sb[0:N1, 1, :])

    nc.compile()
    return nc


def _np_dt(dt):
    import ml_dtypes

    return {
        mybir.dt.float32: np.float32,
        mybir.dt.float32r: np.float32,
        mybir.dt.float16: np.float16,
        mybir.dt.bfloat16: ml_dtypes.bfloat16,
    }[dt]


def prep_inputs(x, qkv_w, q_bias, v_bias, rpb_table, proj_w, proj_b, rel_idx):
    """Host-side staging: shard x over cores, lay out weights for SBUF."""
    x = np.asarray(x, dtype=np.float32)
    qkv_w = np.asarray(qkv_w, dtype=np.float32)
    proj_w = np.asarray(proj_w, dtype=np.float32)
    q_bias = np.asarray(q_bias, dtype=np.float32)
    v_bias = np.asarray(v_bias, dtype=np.float32)
    rpb_table = np.asarray(rpb_table, dtype=np.float32)
    proj_b = np.asarray(proj_b, dtype=np.float32)
    rel_idx = np.asarray(rel_idx)

    big = _np_dt(DT_BIG)
    qkvwT = np.ascontiguousarray(qkv_w.T)  # [768, 2304]
    qkw = np.ascontiguousarray(
        qkvwT[:, : 2 * DIM].reshape(KO, 128, 12, 128).transpose(2, 1, 0, 3)
    ).astype(big)
    vw = np.ascontiguousarray(
        qkvwT[:, 2 * DIM :].reshape(KO, 128, DIM).transpose(1, 0, 2)
    ).astype(big)
    pw = np.ascontiguousarray(
        proj_w.T.reshape(KO, 128, DIM).transpose(1, 0, 2)
    ).astype(big)

    # ebias[mo, mi, h, n] = exp(rpb_table[rel_idx[n, m], h]) with m = mo*128+mi
    bnm = rpb_table[rel_idx]  # [n, m, H]
    bias = np.zeros((2 * 128, H, N), dtype=np.float32)
    bias[:N] = bnm.transpose(1, 2, 0)  # [m, H, n]
    bias = bias.reshape(2, 128, H, N)

    qb = np.zeros((128, 12), dtype=np.float32)
    qb[:, :6] = (q_bias * SCALE).reshape(KO, 128).T
    vb = np.ascontiguousarray(v_bias[None, :])
    pb = np.ascontiguousarray(proj_b[None, :])

    import ml_dtypes

    shared = {
        "qkw": qkw, "vw": vw, "pw": pw, "ebias": np.ascontiguousarray(bias),
        "qb": qb, "vb": vb, "pb": pb,
        "onesw": np.ones((128, 64), dtype=_np_dt(DT_ATT)),
    }
    in_maps = []
    for c in range(CORES):
        xs = x[c * BSH : (c + 1) * BSH]  # [BSH, N, DIM]
        xt = np.ascontiguousarray(
            xs.reshape(BSH * N, DIM).T.reshape(KO, 128, BSH * N).transpose(1, 0, 2)
        ).astype(big)
        in_maps.append({"xt": xt, **shared})
    return in_maps


def _ensure_ntff_hook():
    """Register the axon NTFF profile hook so trace=True yields exec_time_ns.

    The image's antenv package lacks axon_hooks, so boot() degrades silently;
    supply the module via sys.modules and re-register the ctypes hook.
    Best-effort: failure only disables tracing, not execution."""
    import types

    if "antenv.axon_hooks" in sys.modules:
        return
    try:
        mod = types.ModuleType("antenv.axon_hooks")
        _hook = [None]
        mod.set_axon_ntff_profile_hook = lambda h: _hook.__setitem__(0, h)
        mod.get_axon_ntff_profile_hook = lambda: _hook[0]
        from trn_agent_boot.trn_boot import _ntff_profile_via_ctypes

        mod.set_axon_ntff_profile_hook(
            _ntff_profile_via_ctypes("/opt/axon/libaxon_pjrt.so")
        )
        sys.modules["antenv.axon_hooks"] = mod
    except Exception:
        pass


_NC = None


def _get_nc():
    global _NC
    if _NC is None:
        _NC = build_program(BSH)
    return _NC


def kernel(x, qkv_w, q_bias, v_bias, rpb_table, proj_w, proj_b, rel_idx,
           _trace=False, **trace_kwargs):
    if _trace:
        _ensure_ntff_hook()
    nc = _get_nc()
    in_maps = prep_inputs(x, qkv_w, q_bias, v_bias, rpb_table, proj_w, proj_b, rel_idx)
    res = run_bass_kernel_spmd(
        nc, in_maps, core_ids=list(range(CORES)), trace=_trace, **trace_kwargs
    )
    out = np.concatenate([res.results[c]["out"] for c in range(CORES)], axis=0)
    if _trace:
        return out, res
    return out


# revision 27
# speedup vs baseline: 1.0633x; 1.0312x over previous
"""BEiT-style windowed attention block on 8 Trainium2 NeuronCores.

Reference computation (per batch b, head h):
    qkv = x @ qkv_w.T + [q_bias, 0, v_bias]          # [B, N, 3C]
    q, k, v = split(qkv)                              # [B, H, N, D]
    s = (q * D**-0.5) @ k.T + rpb_table[rel_idx].T    # [B, H, N, N]
    p = softmax(s, axis=-1)
    out = (p @ v).reshape(B, N, C) @ proj_w.T + proj_b

Sharding: pure data parallel — batch 64 split as 8 batches per core,
weights + rel-pos-bias table replicated. No collectives.

Device-side layout strategy (per core):
  - x is staged host-side as x^T ("f-major": feature on partitions) so the
    qkv matmuls can use it as the moving operand directly.
  - q^T, k^T are produced f-major ([feat, token]) so the per-head attention
    matmul s^T[m, n] = k^T.T @ q^T needs no transposes.  Softmax runs over
    the partition (m) axis: exp on ACT, denominators via ones-column
    matmuls on the PE, division via a reciprocal row broadcast (DRAM-bounce
    DMA) — softmax is shift-invariant and the scores here are O(1), so the
    max-subtraction is skipped.
  - v is produced token-major ([token, feat]) which is exactly the lhsT
    layout stage-3 (p @ v) wants; its output comes out f-major, which is
    exactly the lhsT layout the final projection wants; the projection
    output comes out token-major, which is what the DMA back to HBM wants.
  - head pairs sit at partition offsets 0/64, so the K=64 / M=64 attention
    matmuls auto-pack into distinct PE row/col groups and run concurrently.
"""

import sys

sys.path.insert(0, "/opt/trn_rl_repo")

import numpy as np

import concourse.bass as bass
import concourse.mybir as mybir
import concourse.tile as tile
from concourse import bacc
from concourse.bass_utils import run_bass_kernel_spmd

F32 = mybir.dt.float32
# Matmul operand dtype. fp16 streams at 1 row/cycle (4x plain fp32's LOW_HIGH
# double-pass), keeps the PE HAM clock warm (unlike float32r, whose datapath
# doesn't register as PE activity and re-throttles the clock to 1.2 GHz), and
# carries 3 more mantissa bits than bf16. All values here are O(100) at most,
# far from fp16 range limits. PSUM accumulation and softmax arithmetic stay
# fp32.
DT_BIG = mybir.dt.float16
DT_ATT = mybir.dt.float16

DIM = 768
H = 12
D = 64
N = 197  # tokens per image
B = 64
CORES = 8
BSH = B // CORES  # batches per core
KO = DIM // 128  # contraction subtiles
SCALE = D ** -0.5
N0, N1 = 128, N - 128  # token chunk sizes (128, 69)


def _act_raw(nc, out, in_, func):
    """InstActivation without bass's accuracy blocklist (used for the
    table-based Reciprocal: measured ~1e-5 rel err, far below the fp16
    matmul noise floor)."""
    eng = nc.scalar
    ins = [eng.lower_ap(in_)]
    for arg in (0.0, 1.0, 0.0):
        ins.append(mybir.ImmediateValue(dtype=mybir.dt.float32, value=arg))
    return eng.add_instruction(
        mybir.InstActivation(
            name=nc.get_next_instruction_name(),
            func=func,
            ins=ins,
            outs=[eng.lower_ap(out)],
        )
    )


def build_program(n_batches: int = BSH):
    nc = bacc.Bacc("TRN2", target_bir_lowering=False, debug=False, num_devices=CORES)

    T = n_batches * N
    xt_d = nc.dram_tensor("xt", [128, KO, T], DT_BIG, kind="ExternalInput")
    qkw_d = nc.dram_tensor("qkw", [12, 128, KO, 128], DT_BIG, kind="ExternalInput")
    vw_d = nc.dram_tensor("vw", [128, KO, DIM], DT_BIG, kind="ExternalInput")
    pw_d = nc.dram_tensor("pw", [128, KO, DIM], DT_BIG, kind="ExternalInput")
    # rel-pos-bias, transposed: bias[mo, mi, h, n] = rpb[rel_idx[n, m], h]
    ebias_d = nc.dram_tensor("ebias", [2, 128, H, N], F32, kind="ExternalInput")
    qb_d = nc.dram_tensor("qb", [128, 12], F32, kind="ExternalInput")
    vb_d = nc.dram_tensor("vb", [1, DIM], F32, kind="ExternalInput")
    pb_d = nc.dram_tensor("pb", [1, DIM], F32, kind="ExternalInput")
    onesw_d = nc.dram_tensor("onesw", [128, 64], DT_ATT, kind="ExternalInput")

    out_d = nc.dram_tensor("out", [n_batches, N, DIM], F32, kind="ExternalOutput")

    with tile.TileContext(nc) as tc:
        with (
            tc.tile_pool(name="wpool", bufs=1) as wpool,
            tc.tile_pool(name="xpool", bufs=2) as xpool,
            tc.tile_pool(name="qkpool", bufs=2) as qkpool,
            tc.tile_pool(name="vpool", bufs=2) as vpool,
            tc.tile_pool(name="eras", bufs=3) as eras,
            tc.tile_pool(name="epool", bufs=3) as epool,
            tc.tile_pool(name="opool", bufs=2) as opool,
            tc.tile_pool(name="otpool", bufs=2) as otpool,
            tc.tile_pool(name="rpool", bufs=2) as rpool,
            tc.tile_pool(name="outpool", bufs=2) as outpool,
            tc.tile_pool(name="ps_mm", bufs=4, space="PSUM") as ps_mm,
            tc.tile_pool(name="ps_s", bufs=2, space="PSUM") as ps_s,
            tc.tile_pool(name="ps_pd", bufs=2, space="PSUM") as ps_pd,
            tc.tile_pool(name="dscr", bufs=2, space="DRAM") as dscr,
        ):
            # ---- persistent weights ----
            qkw = []
            for ft in range(12):
                t = wpool.tile([128, KO, 128], DT_BIG, tag=f"qkw{ft}")
                nc.sync.dma_start(t[:], qkw_d[ft])
                qkw.append(t)
            vw = wpool.tile([128, KO, DIM], DT_BIG, tag="vw")
            nc.sync.dma_start(vw[:], vw_d[:])
            pw = wpool.tile([128, KO, DIM], DT_BIG, tag="pw")
            nc.sync.dma_start(pw[:], pw_d[:])
            ebias = wpool.tile([128, 2, H, N], F32, tag="ebias")
            for mo in range(2):
                nc.sync.dma_start(ebias[:, mo], ebias_d[mo])
            qb = wpool.tile([128, 12], F32, tag="qb")
            nc.sync.dma_start(qb[:], qb_d[:])
            # per-feature biases broadcast across partitions (stride-0 DMA)
            vb_bc = wpool.tile([128, DIM], F32, tag="vb_bc")
            nc.sync.dma_start(
                vb_bc[:],
                bass.AP(tensor=vb_d.ap().tensor, offset=vb_d.ap().offset,
                        ap=[[0, 128]] + list(vb_d.ap().ap[1:])),
            )
            pb_bc = wpool.tile([128, DIM], F32, tag="pb_bc")
            nc.sync.dma_start(
                pb_bc[:],
                bass.AP(tensor=pb_d.ap().tensor, offset=pb_d.ap().offset,
                        ap=[[0, 128]] + list(pb_d.ap().ap[1:])),
            )
            onesw = wpool.tile([128, 64], DT_ATT, tag="onesw")
            nc.sync.dma_start(onesw[:], onesw_d[:])


            assert n_batches % 2 == 0
            for chunk in range(n_batches // 2):
                # ---- load x^T for a 2-batch chunk ----
                xt = xpool.tile([128, KO, 2 * N], DT_BIG, tag="xt")
                nc.sync.dma_start(xt[:], xt_d[:, :, 2 * N * chunk : 2 * N * (chunk + 1)])

                # ---- q^T / k^T (f-major), both batches at once (N=394) ----
                # 456 = 2N + 62 pad cols so the mo=1 score matmul can use a
                # full M=128 stationary slice (rows 69:128 produce scores of
                # neighbouring tokens, initialized but unused).
                qkT = qkpool.tile([128, 12, 456], DT_ATT, tag="qkT")
                nc.vector.memset(qkT[:, :, 2 * N : 456], 0.0)
                for ft in range(12):
                    ps = ps_mm.tile([128, 512], F32, tag="mm")
                    for ko in range(KO):
                        nc.tensor.matmul(
                            ps[:, 0 : 2 * N],
                            qkw[ft][:, ko],
                            xt[:, ko],
                            start=(ko == 0),
                            stop=(ko == KO - 1),
                        )
                    nc.scalar.activation(
                        qkT[:, ft, 0 : 2 * N],
                        ps[:, 0 : 2 * N],
                        mybir.ActivationFunctionType.Identity,
                        bias=qb[:, ft : ft + 1],
                        scale=SCALE if ft < 6 else 1.0,
                    )

                for i in range(2):
                    b = 2 * chunk + i
                    boff = i * N

                    # ---- v (token-major), augmented per head with a ones
                    # half so stage 3 computes output and denominator in one
                    # M=128 matmul.  Even heads: [v | 1], odd heads: [1 | v].
                    v_sb = vpool.tile([128, 2, H, 128], DT_ATT, tag="v")
                    for par in range(2):
                        dst = v_sb.rearrange("p a (g two) c -> p a g two c", two=2)[
                            :, :, :, par, 64 * (1 - par) : 64 * (1 - par) + 64
                        ]
                        osrc = bass.AP(
                            tensor=onesw.tensor, offset=onesw.offset,
                            ap=[list(onesw.ap[0])]
                            + [[0, 2], [0, H // 2]]
                            + [list(onesw.ap[1])],
                        )
                        nc.vector.tensor_copy(dst, osrc)
                    for no, tw in ((0, N0), (1, N1)):
                        for fo, fw in ((0, 512), (512, 256)):
                            psv = ps_mm.tile([128, 512], F32, tag="mm")
                            for ko in range(KO):
                                nc.tensor.matmul(
                                    psv[0:tw, 0:fw],
                                    xt[:, ko, boff + no * 128 : boff + no * 128 + tw],
                                    vw[:, ko, fo : fo + fw],
                                    start=(ko == 0),
                                    stop=(ko == KO - 1),
                                )
                            g = fw // 128
                            hb = fo // 64
                            vsrc = psv[0:tw, 0:fw].rearrange(
                                "p (g two d) -> p g two d", two=2, d=64
                            )
                            vbs = vb_bc[0:tw, fo : fo + fw].rearrange(
                                "p (g two d) -> p g two d", two=2, d=64
                            )
                            vdst = v_sb[0:tw, no, hb : hb + 2 * g, :].rearrange(
                                "p (g two) c -> p g two c", two=2
                            )
                            for par in range(2):
                                nc.vector.tensor_add(
                                    vdst[:, :, par, 64 * par : 64 * par + 64],
                                    vsrc[:, :, par, :],
                                    vbs[:, :, par, :],
                                )

                    # ---- attention, head pairs (2j, 2j+1) ----
                    # The two heads of a pair sit at partition 0 / 64 in the
                    # f-major layouts, so their matmuls land in different PE
                    # row/col groups and run concurrently when adjacent.
                    ohT = opool.tile([128, KO, N], DT_BIG, tag="ohT")
                    out_all = otpool.tile([128, H // 2, N], F32, tag="out_all",
                                          name="out_all")
                    den_stage = otpool.tile([65, H // 2, N], F32, tag="den_stage",
                                            name="den_stage")
                    den_sb = otpool.tile([H, N], F32, tag="den_sb",
                                         name="den_sb")
                    for j in range(H // 2):
                        hA, hB = 2 * j, 2 * j + 1
                        qkTA = qkT[0:64, :, :]
                        qkTB = qkT[64:128, :, :]
                        pssA = ps_s.tile([128, 512], F32, tag="s", name="pssA")
                        pssB = ps_s.tile([128, 512], F32, tag="s", name="pssB")
                        for mo in range(2):
                            psl = slice(mo * N, mo * N + N)
                            nc.tensor.matmul(
                                pssA[:, psl],
                                qkTA[:, 6 + j, boff + 128 * mo : boff + 128 * mo + 128],
                                qkTA[:, j, boff : boff + N],
                                start=True, stop=True,
                            )
                            nc.tensor.matmul(
                                pssB[:, psl],
                                qkTB[:, 6 + j, boff + 128 * mo : boff + 128 * mo + 128],
                                qkTB[:, j, boff : boff + N],
                                start=True, stop=True,
                            )
                        # one DVE add drains the score psum (bias applied,
                        # fp16 out), then exp runs from SBUF on ACT.
                        es_pair = []
                        for pss, h in ((pssA, hA), (pssB, hB)):
                            sa = eras.tile([128, 2, N], DT_ATT, tag="sa")
                            nc.vector.tensor_add(
                                sa[:],
                                pss[:, 0 : 2 * N].rearrange("p (a n) -> p a n", a=2),
                                ebias[:, :, h, :],
                            )
                            es = epool.tile([128, 2, N], DT_ATT, tag="es")
                            nc.scalar.activation(
                                es.rearrange("p a n -> p (a n)"),
                                sa.rearrange("p a n -> p (a n)"),
                                mybir.ActivationFunctionType.Exp,
                            )
                            es_pair.append(es)
                        esA, esB = es_pair

                        # stage 3: one M=128 matmul per head gives out^T rows
                        # and 64 replicated denominator rows in one group.
                        pdA = ps_pd.tile([128, 512], F32, tag="pd", name="pdA")
                        pdB = ps_pd.tile([128, 512], F32, tag="pd", name="pdB")
                        nc.tensor.matmul(pdA[:, 0:N], v_sb[:, 0, hA, :], esA[:, 0, :],
                                         start=True, stop=False)
                        nc.tensor.matmul(pdB[:, 0:N], v_sb[:, 0, hB, :], esB[:, 0, :],
                                         start=True, stop=False)
                        nc.tensor.matmul(pdA[:, 0:N], v_sb[0:N1, 1, hA, :],
                                         esA[0:N1, 1, :], start=False, stop=True)
                        nc.tensor.matmul(pdB[:, 0:N], v_sb[0:N1, 1, hB, :],
                                         esB[0:N1, 1, :], start=False, stop=True)

                        # even head: out rows 0:64, den (replicated) 64:128;
                        # odd head: den rows 0:64, out rows 64:128.
                        nc.scalar.activation(
                            out_all[0:64, j, :], pdA[0:64, 0:N],
                            mybir.ActivationFunctionType.Copy,
                        )
                        nc.scalar.activation(
                            out_all[64:128, j, :], pdB[64:128, 0:N],
                            mybir.ActivationFunctionType.Copy,
                        )
                        # single denominator rows leave psum via tiny legal-
                        # base DVE copies, then SBUF DMAs pack them to 12 rows.
                        nc.vector.tensor_copy(den_stage[64:65, j, :],
                                              pdA[64:65, 0:N])
                        nc.vector.tensor_copy(den_stage[0:1, j, :],
                                              pdB[0:1, 0:N])
                        nc.sync.dma_start(den_sb[2 * j : 2 * j + 1, :],
                                          den_stage[64:65, j, :])
                        nc.sync.dma_start(den_sb[2 * j + 1 : 2 * j + 2, :],
                                          den_stage[0:1, j, :])

                    # softmax division: one small reciprocal per batch, then
                    # broadcast each head's row over its 64 output partitions
                    # via a DRAM bounce, and one elementwise multiply.
                    rvc = rpool.tile([H, N], F32, tag="rvc")
                    nc.vector.reciprocal(rvc[:], den_sb[:])
                    scr = dscr.tile([H, N], F32, tag="scr")
                    nc.sync.dma_start(scr[:], rvc[:])
                    rv2 = rpool.tile([128, H // 2, N], F32, tag="rv2")
                    for j in range(H // 2):
                        for rows, hh in ((slice(0, 64), 0), (slice(64, 128), 1)):
                            s_ap = scr[2 * j + hh, :][None]
                            bsrc = bass.AP(
                                tensor=s_ap.tensor, offset=s_ap.offset,
                                ap=[[0, 64]] + list(s_ap.ap[1:]),
                            )
                            nc.sync.dma_start(rv2[rows, j, :], bsrc)
                    nc.gpsimd.tensor_mul(ohT[:], out_all[:], rv2[:])

                    # ---- projection (token-major out) + bias ----
                    out_sb = outpool.tile([128, 2, DIM], F32, tag="out")
                    for no, tw in ((0, N0), (1, N1)):
                        for fo, fw in ((0, 512), (512, 256)):
                            psp = ps_mm.tile([128, 512], F32, tag="mm")
                            for ko in range(KO):
                                nc.tensor.matmul(
                                    psp[0:tw, 0:fw],
                                    ohT[:, ko, no * 128 : no * 128 + tw],
                                    pw[:, ko, fo : fo + fw],
                                    start=(ko == 0),
                                    stop=(ko == KO - 1),
                                )
                            nc.vector.tensor_add(
                                out_sb[0:tw, no, fo : fo + fw],
                                psp[0:tw, 0:fw],
                                pb_bc[0:tw, fo : fo + fw],
                            )
                    nc.sync.dma_start(out_d[b, 0:128, :], out_sb[:, 0, :])
                    nc.sync.dma_start(out_d[b, 128:N, :], out_sb[0:N1, 1, :])

    nc.compile()
    return nc


def _np_dt(dt):
    import ml_dtypes

    return {
        mybir.dt.float32: np.float32,
        mybir.dt.float32r: np.float32,
        mybir.dt.float16: np.float16,
        mybir.dt.bfloat16: ml_dtypes.bfloat16,
    }[dt]


def prep_inputs(x, qkv_w, q_bias, v_bias, rpb_table, proj_w, proj_b, rel_idx):
    """Host-side staging: shard x over cores, lay out weights for SBUF."""
    x = np.asarray(x, dtype=np.float32)
    qkv_w = np.asarray(qkv_w, dtype=np.float32)
    proj_w = np.asarray(proj_w, dtype=np.float32)
    q_bias = np.asarray(q_bias, dtype=np.float32)
    v_bias = np.asarray(v_bias, dtype=np.float32)
    rpb_table = np.asarray(rpb_table, dtype=np.float32)
    proj_b = np.asarray(proj_b, dtype=np.float32)
    rel_idx = np.asarray(rel_idx)

    big = _np_dt(DT_BIG)
    qkvwT = np.ascontiguousarray(qkv_w.T)  # [768, 2304]
    qkw = np.ascontiguousarray(
        qkvwT[:, : 2 * DIM].reshape(KO, 128, 12, 128).transpose(2, 1, 0, 3)
    ).astype(big)
    vw = np.ascontiguousarray(
        qkvwT[:, 2 * DIM :].reshape(KO, 128, DIM).transpose(1, 0, 2)
    ).astype(big)
    pw = np.ascontiguousarray(
        proj_w.T.reshape(KO, 128, DIM).transpose(1, 0, 2)
    ).astype(big)

    # ebias[mo, mi, h, n] = exp(rpb_table[rel_idx[n, m], h]) with m = mo*128+mi
    bnm = rpb_table[rel_idx]  # [n, m, H]
    bias = np.zeros((2 * 128, H, N), dtype=np.float32)
    bias[:N] = bnm.transpose(1, 2, 0)  # [m, H, n]
    bias = bias.reshape(2, 128, H, N)

    qb = np.zeros((128, 12), dtype=np.float32)
    qb[:, :6] = (q_bias * SCALE).reshape(KO, 128).T
    vb = np.ascontiguousarray(v_bias[None, :])
    pb = np.ascontiguousarray(proj_b[None, :])

    import ml_dtypes

    shared = {
        "qkw": qkw, "vw": vw, "pw": pw, "ebias": np.ascontiguousarray(bias),
        "qb": qb, "vb": vb, "pb": pb,
        "onesw": np.ones((128, 64), dtype=_np_dt(DT_ATT)),
    }
    in_maps = []
    for c in range(CORES):
        xs = x[c * BSH : (c + 1) * BSH]  # [BSH, N, DIM]
        xt = np.ascontiguousarray(
            xs.reshape(BSH * N, DIM).T.reshape(KO, 128, BSH * N).transpose(1, 0, 2)
        ).astype(big)
        in_maps.append({"xt": xt, **shared})
    return in_maps


def _ensure_ntff_hook():
    """Register the axon NTFF profile hook so trace=True yields exec_time_ns.

    The image's antenv package lacks axon_hooks, so boot() degrades silently;
    supply the module via sys.modules and re-register the ctypes hook.
    Best-effort: failure only disables tracing, not execution."""
    import types

    if "antenv.axon_hooks" in sys.modules:
        return
    try:
        mod = types.ModuleType("antenv.axon_hooks")
        _hook = [None]
        mod.set_axon_ntff_profile_hook = lambda h: _hook.__setitem__(0, h)
        mod.get_axon_ntff_profile_hook = lambda: _hook[0]
        from trn_agent_boot.trn_boot import _ntff_profile_via_ctypes

        mod.set_axon_ntff_profile_hook(
            _ntff_profile_via_ctypes("/opt/axon/libaxon_pjrt.so")
        )
        sys.modules["antenv.axon_hooks"] = mod
    except Exception:
        pass


_NC = None


def _get_nc():
    global _NC
    if _NC is None:
        _NC = build_program(BSH)
    return _NC


def kernel(x, qkv_w, q_bias, v_bias, rpb_table, proj_w, proj_b, rel_idx,
           _trace=False, **trace_kwargs):
    if _trace:
        _ensure_ntff_hook()
    nc = _get_nc()
    in_maps = prep_inputs(x, qkv_w, q_bias, v_bias, rpb_table, proj_w, proj_b, rel_idx)
    res = run_bass_kernel_spmd(
        nc, in_maps, core_ids=list(range(CORES)), trace=_trace, **trace_kwargs
    )
    out = np.concatenate([res.results[c]["out"] for c in range(CORES)], axis=0)
    if _trace:
        return out, res
    return out


# revision 29
# speedup vs baseline: 1.1679x; 1.0983x over previous
"""BEiT-style windowed attention block on 8 Trainium2 NeuronCores.

Reference computation (per batch b, head h):
    qkv = x @ qkv_w.T + [q_bias, 0, v_bias]          # [B, N, 3C]
    q, k, v = split(qkv)                              # [B, H, N, D]
    s = (q * D**-0.5) @ k.T + rpb_table[rel_idx].T    # [B, H, N, N]
    p = softmax(s, axis=-1)
    out = (p @ v).reshape(B, N, C) @ proj_w.T + proj_b

Sharding: pure data parallel — batch 64 split as 8 batches per core,
weights + rel-pos-bias table replicated. No collectives.

Device-side layout strategy (per core):
  - x is staged host-side as x^T ("f-major": feature on partitions) so the
    qkv matmuls can use it as the moving operand directly.
  - q^T, k^T are produced f-major ([feat, token]) so the per-head attention
    matmul s^T[m, n] = k^T.T @ q^T needs no transposes.  Softmax runs over
    the partition (m) axis: exp on ACT, denominators via ones-column
    matmuls on the PE, division via a reciprocal row broadcast (DRAM-bounce
    DMA) — softmax is shift-invariant and the scores here are O(1), so the
    max-subtraction is skipped.
  - v is produced token-major ([token, feat]) which is exactly the lhsT
    layout stage-3 (p @ v) wants; its output comes out f-major, which is
    exactly the lhsT layout the final projection wants; the projection
    output comes out token-major, which is what the DMA back to HBM wants.
  - head pairs sit at partition offsets 0/64, so the K=64 / M=64 attention
    matmuls auto-pack into distinct PE row/col groups and run concurrently.
"""

import sys

sys.path.insert(0, "/opt/trn_rl_repo")

import numpy as np

import concourse.bass as bass
import concourse.mybir as mybir
import concourse.tile as tile
from concourse import bacc
from concourse.bass_utils import run_bass_kernel_spmd

F32 = mybir.dt.float32
# Matmul operand dtype. fp16 streams at 1 row/cycle (4x plain fp32's LOW_HIGH
# double-pass), keeps the PE HAM clock warm (unlike float32r, whose datapath
# doesn't register as PE activity and re-throttles the clock to 1.2 GHz), and
# carries 3 more mantissa bits than bf16. All values here are O(100) at most,
# far from fp16 range limits. PSUM accumulation and softmax arithmetic stay
# fp32.
DT_BIG = mybir.dt.float16
DT_ATT = mybir.dt.float16

DIM = 768
H = 12
D = 64
N = 197  # tokens per image
B = 64
CORES = 8
BSH = B // CORES  # batches per core
KO = DIM // 128  # contraction subtiles
SCALE = D ** -0.5
N0, N1 = 128, N - 128  # token chunk sizes (128, 69)


def _act_raw(nc, out, in_, func):
    """InstActivation without bass's accuracy blocklist (used for the
    table-based Reciprocal: measured ~1e-5 rel err, far below the fp16
    matmul noise floor)."""
    eng = nc.scalar
    ins = [eng.lower_ap(in_)]
    for arg in (0.0, 1.0, 0.0):
        ins.append(mybir.ImmediateValue(dtype=mybir.dt.float32, value=arg))
    return eng.add_instruction(
        mybir.InstActivation(
            name=nc.get_next_instruction_name(),
            func=func,
            ins=ins,
            outs=[eng.lower_ap(out)],
        )
    )


def build_program(n_batches: int = BSH):
    nc = bacc.Bacc("TRN2", target_bir_lowering=False, debug=False, num_devices=CORES)

    T = n_batches * N
    xt_d = nc.dram_tensor("xt", [128, KO, T], DT_BIG, kind="ExternalInput")
    qkw_d = nc.dram_tensor("qkw", [12, 128, KO, 128], DT_BIG, kind="ExternalInput")
    vw_d = nc.dram_tensor("vw", [128, KO, DIM], DT_BIG, kind="ExternalInput")
    pw_d = nc.dram_tensor("pw", [128, KO, DIM], DT_BIG, kind="ExternalInput")
    # rel-pos-bias, transposed: bias[mo, mi, h, n] = rpb[rel_idx[n, m], h]
    ebias_d = nc.dram_tensor("ebias", [2, 128, H, N], F32, kind="ExternalInput")
    qb_d = nc.dram_tensor("qb", [128, 12], F32, kind="ExternalInput")
    vb_d = nc.dram_tensor("vb", [1, DIM], F32, kind="ExternalInput")
    pb_d = nc.dram_tensor("pb", [1, DIM], F32, kind="ExternalInput")
    onesw_d = nc.dram_tensor("onesw", [128, 64], DT_ATT, kind="ExternalInput")

    out_d = nc.dram_tensor("out", [n_batches, N, DIM], F32, kind="ExternalOutput")

    with tile.TileContext(nc) as tc:
        with (
            tc.tile_pool(name="wpool", bufs=1) as wpool,
            tc.tile_pool(name="xpool", bufs=2) as xpool,
            tc.tile_pool(name="qkpool", bufs=2) as qkpool,
            tc.tile_pool(name="vpool", bufs=2) as vpool,
            tc.tile_pool(name="eras", bufs=3) as eras,
            tc.tile_pool(name="epool", bufs=3) as epool,
            tc.tile_pool(name="opool", bufs=2) as opool,
            tc.tile_pool(name="otpool", bufs=2) as otpool,
            tc.tile_pool(name="rpool", bufs=2) as rpool,
            tc.tile_pool(name="outpool", bufs=2) as outpool,
            tc.tile_pool(name="ps_mm", bufs=4, space="PSUM") as ps_mm,
            tc.tile_pool(name="ps_s", bufs=2, space="PSUM") as ps_s,
            tc.tile_pool(name="ps_pd", bufs=2, space="PSUM") as ps_pd,
            tc.tile_pool(name="dscr", bufs=2, space="DRAM") as dscr,
        ):
            # ---- persistent weights ----
            qkw = []
            for ft in range(12):
                t = wpool.tile([128, KO, 128], DT_BIG, tag=f"qkw{ft}")
                nc.sync.dma_start(t[:], qkw_d[ft])
                qkw.append(t)
            vw = wpool.tile([128, KO, DIM], DT_BIG, tag="vw")
            nc.sync.dma_start(vw[:], vw_d[:])
            pw = wpool.tile([128, KO, DIM], DT_BIG, tag="pw")
            nc.sync.dma_start(pw[:], pw_d[:])
            ebias = wpool.tile([128, 2, H, N], F32, tag="ebias")
            for mo in range(2):
                nc.sync.dma_start(ebias[:, mo], ebias_d[mo])
            qb = wpool.tile([128, 12], F32, tag="qb")
            nc.sync.dma_start(qb[:], qb_d[:])
            # per-feature biases broadcast across partitions (stride-0 DMA)
            vb_bc = wpool.tile([128, DIM], F32, tag="vb_bc")
            nc.sync.dma_start(
                vb_bc[:],
                bass.AP(tensor=vb_d.ap().tensor, offset=vb_d.ap().offset,
                        ap=[[0, 128]] + list(vb_d.ap().ap[1:])),
            )
            pb_bc = wpool.tile([128, DIM], F32, tag="pb_bc")
            nc.sync.dma_start(
                pb_bc[:],
                bass.AP(tensor=pb_d.ap().tensor, offset=pb_d.ap().offset,
                        ap=[[0, 128]] + list(pb_d.ap().ap[1:])),
            )
            onesw = wpool.tile([128, 64], DT_ATT, tag="onesw")
            nc.sync.dma_start(onesw[:], onesw_d[:])


            assert n_batches % 2 == 0
            for chunk in range(n_batches // 2):
                # ---- load x^T for a 2-batch chunk ----
                xt = xpool.tile([128, KO, 2 * N], DT_BIG, tag="xt")
                nc.sync.dma_start(xt[:], xt_d[:, :, 2 * N * chunk : 2 * N * (chunk + 1)])

                # ---- q^T / k^T (f-major), both batches at once (N=394) ----
                # 456 = 2N + 62 pad cols so the mo=1 score matmul can use a
                # full M=128 stationary slice (rows 69:128 produce scores of
                # neighbouring tokens, initialized but unused).
                qkT = qkpool.tile([128, 12, 456], DT_ATT, tag="qkT")
                nc.vector.memset(qkT[:, :, 2 * N : 456], 0.0)
                for ft in range(12):
                    ps = ps_mm.tile([128, 512], F32, tag="mm")
                    for ko in range(KO):
                        nc.tensor.matmul(
                            ps[:, 0 : 2 * N],
                            qkw[ft][:, ko],
                            xt[:, ko],
                            start=(ko == 0),
                            stop=(ko == KO - 1),
                        )
                    nc.scalar.activation(
                        qkT[:, ft, 0 : 2 * N],
                        ps[:, 0 : 2 * N],
                        mybir.ActivationFunctionType.Identity,
                        bias=qb[:, ft : ft + 1],
                        scale=SCALE if ft < 6 else 1.0,
                    )

                for i in range(2):
                    b = 2 * chunk + i
                    boff = i * N

                    # ---- v (token-major), augmented per head with a ones
                    # half so stage 3 computes output and denominator in one
                    # M=128 matmul.  Even heads: [v | 1], odd heads: [1 | v].
                    v_sb = vpool.tile([128, 2, H, 128], DT_ATT, tag="v")
                    for par in range(2):
                        dst = v_sb.rearrange("p a (g two) c -> p a g two c", two=2)[
                            :, :, :, par, 64 * (1 - par) : 64 * (1 - par) + 64
                        ]
                        osrc = bass.AP(
                            tensor=onesw.tensor, offset=onesw.offset,
                            ap=[list(onesw.ap[0])]
                            + [[0, 2], [0, H // 2]]
                            + [list(onesw.ap[1])],
                        )
                        nc.vector.tensor_copy(dst, osrc)
                    for no, tw in ((0, N0), (1, N1)):
                        for fo, fw in ((0, 512), (512, 256)):
                            psv = ps_mm.tile([128, 512], F32, tag="mm")
                            for ko in range(KO):
                                nc.tensor.matmul(
                                    psv[0:tw, 0:fw],
                                    xt[:, ko, boff + no * 128 : boff + no * 128 + tw],
                                    vw[:, ko, fo : fo + fw],
                                    start=(ko == 0),
                                    stop=(ko == KO - 1),
                                )
                            g = fw // 128
                            hb = fo // 64
                            vsrc = psv[0:tw, 0:fw].rearrange(
                                "p (g two d) -> p g two d", two=2, d=64
                            )
                            vbs = vb_bc[0:tw, fo : fo + fw].rearrange(
                                "p (g two d) -> p g two d", two=2, d=64
                            )
                            vdst = v_sb[0:tw, no, hb : hb + 2 * g, :].rearrange(
                                "p (g two) c -> p g two c", two=2
                            )
                            for par in range(2):
                                nc.vector.tensor_add(
                                    vdst[:, :, par, 64 * par : 64 * par + 64],
                                    vsrc[:, :, par, :],
                                    vbs[:, :, par, :],
                                )

                    # ---- attention, head pairs (2j, 2j+1) ----
                    # The two heads of a pair sit at partition 0 / 64 in the
                    # f-major layouts, so their matmuls land in different PE
                    # row/col groups and run concurrently when adjacent.
                    ohT = opool.tile([128, KO, N], DT_BIG, tag="ohT")
                    out_all = otpool.tile([128, H // 2, N], F32, tag="out_all",
                                          name="out_all")
                    den_stage = otpool.tile([65, H // 2, N], F32, tag="den_stage",
                                            name="den_stage")
                    den_sb = otpool.tile([H, N], F32, tag="den_sb",
                                         name="den_sb")
                    for j in range(H // 2):
                        hA, hB = 2 * j, 2 * j + 1
                        qkTA = qkT[0:64, :, :]
                        qkTB = qkT[64:128, :, :]
                        pssA = ps_s.tile([128, 512], F32, tag="s", name="pssA")
                        pssB = ps_s.tile([128, 512], F32, tag="s", name="pssB")
                        for mo in range(2):
                            psl = slice(mo * N, mo * N + N)
                            nc.tensor.matmul(
                                pssA[:, psl],
                                qkTA[:, 6 + j, boff + 128 * mo : boff + 128 * mo + 128],
                                qkTA[:, j, boff : boff + N],
                                start=True, stop=True,
                            )
                            nc.tensor.matmul(
                                pssB[:, psl],
                                qkTB[:, 6 + j, boff + 128 * mo : boff + 128 * mo + 128],
                                qkTB[:, j, boff : boff + N],
                                start=True, stop=True,
                            )
                        # one DVE add drains the score psum (bias applied,
                        # fp16 out), then exp runs from SBUF on ACT.
                        es_pair = []
                        for pss, h in ((pssA, hA), (pssB, hB)):
                            sa = eras.tile([128, 2, N], DT_ATT, tag="sa")
                            nc.vector.tensor_add(
                                sa[:],
                                pss[:, 0 : 2 * N].rearrange("p (a n) -> p a n", a=2),
                                ebias[:, :, h, :],
                            )
                            es = epool.tile([128, 2, N], DT_ATT, tag="es")
                            nc.scalar.activation(
                                es.rearrange("p a n -> p (a n)"),
                                sa.rearrange("p a n -> p (a n)"),
                                mybir.ActivationFunctionType.Exp,
                            )
                            es_pair.append(es)
                        esA, esB = es_pair

                        # stage 3: one M=128 matmul per head gives out^T rows
                        # and 64 replicated denominator rows in one group.
                        pdA = ps_pd.tile([128, 512], F32, tag="pd", name="pdA")
                        pdB = ps_pd.tile([128, 512], F32, tag="pd", name="pdB")
                        nc.tensor.matmul(pdA[:, 0:N], v_sb[:, 0, hA, :], esA[:, 0, :],
                                         start=True, stop=False)
                        nc.tensor.matmul(pdB[:, 0:N], v_sb[:, 0, hB, :], esB[:, 0, :],
                                         start=True, stop=False)
                        nc.tensor.matmul(pdA[:, 0:N], v_sb[0:N1, 1, hA, :],
                                         esA[0:N1, 1, :], start=False, stop=True)
                        nc.tensor.matmul(pdB[:, 0:N], v_sb[0:N1, 1, hB, :],
                                         esB[0:N1, 1, :], start=False, stop=True)

                        # even head: out rows 0:64, den (replicated) 64:128;
                        # odd head: den rows 0:64, out rows 64:128.
                        nc.scalar.activation(
                            out_all[0:64, j, :], pdA[0:64, 0:N],
                            mybir.ActivationFunctionType.Copy,
                        )
                        nc.scalar.activation(
                            out_all[64:128, j, :], pdB[64:128, 0:N],
                            mybir.ActivationFunctionType.Copy,
                        )
                        # single denominator rows leave psum via tiny legal-
                        # base DVE copies, then SBUF DMAs pack them to 12 rows.
                        nc.vector.tensor_copy(den_stage[64:65, j, :],
                                              pdA[64:65, 0:N])
                        nc.vector.tensor_copy(den_stage[0:1, j, :],
                                              pdB[0:1, 0:N])


                    # softmax division: two strided DMAs pack the 12 head
                    # denominator rows to [12, N], one small reciprocal, one
                    # DRAM bounce, two strided broadcast DMAs, one multiply.
                    nc.sync.dma_start(den_sb[0 : H // 2, :], den_stage[64:65, :, :])
                    nc.sync.dma_start(den_sb[H // 2 : H, :], den_stage[0:1, :, :])
                    rvc = rpool.tile([H, N], F32, tag="rvc")
                    nc.vector.reciprocal(rvc[:], den_sb[:])
                    scr = dscr.tile([H, N], F32, tag="scr")
                    nc.sync.dma_start(scr[:], rvc[:])
                    rv2 = rpool.tile([128, H // 2, N], F32, tag="rv2")
                    scr_ap = scr[:].rearrange("h n -> (h n)")
                    for rows, off in ((slice(0, 64), 0), (slice(64, 128), H // 2 * N)):
                        bsrc = bass.AP(
                            tensor=scr_ap.tensor, offset=scr_ap.offset + off,
                            ap=[[0, 64], [N, H // 2], [1, N]],
                        )
                        nc.sync.dma_start(rv2[rows, :, :], bsrc)
                    nc.gpsimd.tensor_mul(ohT[:], out_all[:], rv2[:])

                    # ---- projection (token-major out) + bias ----
                    out_sb = outpool.tile([128, 2, DIM], F32, tag="out")
                    for no, tw in ((0, N0), (1, N1)):
                        for fo, fw in ((0, 512), (512, 256)):
                            psp = ps_mm.tile([128, 512], F32, tag="mm")
                            for ko in range(KO):
                                nc.tensor.matmul(
                                    psp[0:tw, 0:fw],
                                    ohT[:, ko, no * 128 : no * 128 + tw],
                                    pw[:, ko, fo : fo + fw],
                                    start=(ko == 0),
                                    stop=(ko == KO - 1),
                                )
                            nc.vector.tensor_add(
                                out_sb[0:tw, no, fo : fo + fw],
                                psp[0:tw, 0:fw],
                                pb_bc[0:tw, fo : fo + fw],
                            )
                    nc.sync.dma_start(out_d[b, 0:128, :], out_sb[:, 0, :])
                    nc.sync.dma_start(out_d[b, 128:N, :], out_sb[0:N1, 1, :])

    nc.compile()
    return nc


def _np_dt(dt):
    import ml_dtypes

    return {
        mybir.dt.float32: np.float32,
        mybir.dt.float32r: np.float32,
        mybir.dt.float16: np.float16,
        mybir.dt.bfloat16: ml_dtypes.bfloat16,
    }[dt]


def prep_inputs(x, qkv_w, q_bias, v_bias, rpb_table, proj_w, proj_b, rel_idx):
    """Host-side staging: shard x over cores, lay out weights for SBUF."""
    x = np.asarray(x, dtype=np.float32)
    qkv_w = np.asarray(qkv_w, dtype=np.float32)
    proj_w = np.asarray(proj_w, dtype=np.float32)
    q_bias = np.asarray(q_bias, dtype=np.float32)
    v_bias = np.asarray(v_bias, dtype=np.float32)
    rpb_table = np.asarray(rpb_table, dtype=np.float32)
    proj_b = np.asarray(proj_b, dtype=np.float32)
    rel_idx = np.asarray(rel_idx)

    big = _np_dt(DT_BIG)
    qkvwT = np.ascontiguousarray(qkv_w.T)  # [768, 2304]
    qkw = np.ascontiguousarray(
        qkvwT[:, : 2 * DIM].reshape(KO, 128, 12, 128).transpose(2, 1, 0, 3)
    ).astype(big)
    vw = np.ascontiguousarray(
        qkvwT[:, 2 * DIM :].reshape(KO, 128, DIM).transpose(1, 0, 2)
    ).astype(big)
    pw = np.ascontiguousarray(
        proj_w.T.reshape(KO, 128, DIM).transpose(1, 0, 2)
    ).astype(big)

    # ebias[mo, mi, h, n] = exp(rpb_table[rel_idx[n, m], h]) with m = mo*128+mi
    bnm = rpb_table[rel_idx]  # [n, m, H]
    bias = np.zeros((2 * 128, H, N), dtype=np.float32)
    bias[:N] = bnm.transpose(1, 2, 0)  # [m, H, n]
    bias = bias.reshape(2, 128, H, N)

    qb = np.zeros((128, 12), dtype=np.float32)
    qb[:, :6] = (q_bias * SCALE).reshape(KO, 128).T
    vb = np.ascontiguousarray(v_bias[None, :])
    pb = np.ascontiguousarray(proj_b[None, :])

    import ml_dtypes

    shared = {
        "qkw": qkw, "vw": vw, "pw": pw, "ebias": np.ascontiguousarray(bias),
        "qb": qb, "vb": vb, "pb": pb,
        "onesw": np.ones((128, 64), dtype=_np_dt(DT_ATT)),
    }
    in_maps = []
    for c in range(CORES):
        xs = x[c * BSH : (c + 1) * BSH]  # [BSH, N, DIM]
        xt = np.ascontiguousarray(
            xs.reshape(BSH * N, DIM).T.reshape(KO, 128, BSH * N).transpose(1, 0, 2)
        ).astype(big)
        in_maps.append({"xt": xt, **shared})
    return in_maps


def _ensure_ntff_hook():
    """Register the axon NTFF profile hook so trace=True yields exec_time_ns.

    The image's antenv package lacks axon_hooks, so boot() degrades silently;
    supply the module via sys.modules and re-register the ctypes hook.
    Best-effort: failure only disables tracing, not execution."""
    import types

    if "antenv.axon_hooks" in sys.modules:
        return
    try:
        mod = types.ModuleType("antenv.axon_hooks")
        _hook = [None]
        mod.set_axon_ntff_profile_hook = lambda h: _hook.__setitem__(0, h)
        mod.get_axon_ntff_profile_hook = lambda: _hook[0]
        from trn_agent_boot.trn_boot import _ntff_profile_via_ctypes

        mod.set_axon_ntff_profile_hook(
            _ntff_profile_via_ctypes("/opt/axon/libaxon_pjrt.so")
        )
        sys.modules["antenv.axon_hooks"] = mod
    except Exception:
        pass


_NC = None


def _get_nc():
    global _NC
    if _NC is None:
        _NC = build_program(BSH)
    return _NC


def kernel(x, qkv_w, q_bias, v_bias, rpb_table, proj_w, proj_b, rel_idx,
           _trace=False, **trace_kwargs):
    if _trace:
        _ensure_ntff_hook()
    nc = _get_nc()
    in_maps = prep_inputs(x, qkv_w, q_bias, v_bias, rpb_table, proj_w, proj_b, rel_idx)
    res = run_bass_kernel_spmd(
        nc, in_maps, core_ids=list(range(CORES)), trace=_trace, **trace_kwargs
    )
    out = np.concatenate([res.results[c]["out"] for c in range(CORES)], axis=0)
    if _trace:
        return out, res
    return out
